# revision 1
# baseline (speedup 1.0000x reference)
"""MultiHeadAttention (B=2, S=2048, D=1024, H=16, dk=dv=64) on 8 trn2 cores.

Head-parallel: core c owns heads (2c, 2c+1). The reference's odd
reshape(B,-1,H*DV) means output row m draws only from head m//256, so the
final fc is fully local per core; host just concatenates.

Math transformations (exact, softmax-invariant):
  - bk dropped: adds a per-query constant to scores -> softmax unchanged.
  - bv folded into the output bias: softmax rows sum to 1, so
    att = w@v0 + bv; downstream y += tile16(bv) @ Wo, precomputed on host.
  - softmax without max-subtraction: scores ~ N(0,1), exp is safe in fp32.

Device dataflow per core (fp32r matmuls = full-rate at N>=512):
  phase A (per b): qT/kT/vT [128=2x64 feats, 2048] = W.T @ actT, bq added.
  phase A'(per b): v -> natural [t,f] via PE transposes + ones column.
  phase B (per b, per 512-wide s-quarter): for each 128-wide t-tile:
     scoresT[t, s] for both heads into one [128,1024] psum; one Exp ACT op;
     attT[65,512] += v_aug.T @ exp  (row 64 = softmax denominator).
     Then normalize: reciprocal(row64) -> K=1 broadcast matmul -> multiply.
  phase C: y[m, o] = sum_j attT[:, j::16].T @ Wo[64j:64j+64, o] + bias-mm.
"""

import numpy as np

import concourse.bacc as bacc
import concourse.mybir as mybir
import concourse.tile as tile

B, S, D, H, DK = 2, 2048, 1024, 16, 64
NCORES = 8
KT = D // 128  # 8 contraction tiles
TT = S // 128  # 16 t-tiles
SQ = S // 512  # 4 s-quarters
F32R = mybir.dt.float32r
F32 = mybir.dt.float32
F16 = mybir.dt.float16
USE_FP16 = True
AD = F16 if USE_FP16 else F32R  # dtype of the activation/weight stream
NP_AD = np.float16 if USE_FP16 else np.float32
AF = mybir.ActivationFunctionType


def build_nc(reps=1):
    nc = bacc.Bacc(trn_type="TRN2")

    qT = nc.declare_dram_parameter("qT", [B, KT, 128, S], AD, isOutput=False)
    kTd = nc.declare_dram_parameter("kT", [B, KT, 128, S], AD, isOutput=False)
    vTd = nc.declare_dram_parameter("vT", [B, KT, 128, S], AD, isOutput=False)
    wq = nc.declare_dram_parameter("wq", [128, KT, 128], AD, isOutput=False)
    wk = nc.declare_dram_parameter("wk", [128, KT, 128], AD, isOutput=False)
    wv = nc.declare_dram_parameter("wv", [128, KT, 128], AD, isOutput=False)
    bqd = nc.declare_dram_parameter("bq", [128, 1], F32, isOutput=False)
    eyed = nc.declare_dram_parameter("eye", [128, 64], AD, isOutput=False)
    onesd = nc.declare_dram_parameter("ones", [128, 128], AD, isOutput=False)
    wo = nc.declare_dram_parameter("wo", [16, 64, 1024], AD, isOutput=False)
    bo2 = nc.declare_dram_parameter("bo2", [2, 1024], AD, isOutput=False)
    y = nc.declare_dram_parameter("y", [2, B, 128, 1024], F32R, isOutput=True)

    with tile.TileContext(nc) as tc:
        with (
            tc.tile_pool(name="const", bufs=1) as constp,
            tc.tile_pool(name="wts", bufs=1) as wtsp,
            tc.tile_pool(name="acts", bufs=3) as actsp,
            tc.tile_pool(name="proj", bufs=2) as projp,
            tc.tile_pool(name="vaugp", bufs=2) as vaugp,
            tc.tile_pool(name="exp", bufs=3) as expp,
            tc.tile_pool(name="attp", bufs=4) as attp,
            tc.tile_pool(name="small", bufs=2) as smallp,
            tc.tile_pool(name="wop", bufs=6) as wop,
            tc.tile_pool(name="ysbp", bufs=4) as ysbp,
            tc.tile_pool(name="ps", bufs=1, space="PSUM") as ps,
        ):
            # constants
            ident = constp.tile([128, 64], AD, tag="ident")
            nc.sync.dma_start(out=ident, in_=eyed[:, :])
            ones_sb = constp.tile([128, 128], AD, tag="ones_sb")
            nc.sync.dma_start(out=ones_sb, in_=onesd[:, :])
            ones1 = ones_sb
            bq_sb = constp.tile([128, 1], F32, tag="bq")
            nc.sync.dma_start(out=bq_sb, in_=bqd[:, :])
            bo_sb = constp.tile([1, 2, 1024], AD, tag="bo")
            nc.sync.dma_start(out=bo_sb, in_=bo2[None, :, :])

            # packed per-head weights, resident
            w_sb = {}
            for name, dram in (("q", wq), ("k", wk), ("v", wv)):
                w_sb[name] = wtsp.tile([128, KT, 128], AD, tag="w" + name, name="w" + name)
                nc.sync.dma_start(out=w_sb[name], in_=dram[:, :, :])

            for rep in range(reps):
                attTs = {}  # (hl, b) -> [65, S]: rows 0-63 unnormalized attT,
                # row 64 = reciprocal of the softmax denominator (after B)
                for hl in range(2):
                    for b in range(B):
                        attTs[(hl, b)] = attp.tile([65, S], AD, tag="attT", name=f"attT{hl}{b}")

                for b in range(B):
                    # ---------------- phase A: projections ----------------
                    proj = {}
                    dma_eng = {"q": nc.sync, "k": nc.scalar, "v": nc.gpsimd}
                    for name, dram in (("q", qT), ("k", kTd), ("v", vTd)):
                        dst = projp.tile([128, S], AD, tag=name + "t", name=name + "t")
                        for sh in range(2):
                            pjs = [
                                ps.tile([128, 512], F32, tag="pj", bufs=2, name=f"pj{i}")
                                for i in range(2)
                            ]
                            for k in range(KT):
                                a = actsp.tile(
                                    [128, 1024], AD, tag="a" + name, name="a" + name
                                )
                                dma_eng[name].dma_start(
                                    out=a, in_=dram[b, k, :, sh * 1024 : (sh + 1) * 1024]
                                )
                                for i in range(2):
                                    nc.tensor.matmul(
                                        pjs[i],
                                        w_sb[name][:, k, :],
                                        a[:, i * 512 : (i + 1) * 512],
                                        start=(k == 0),
                                        stop=(k == KT - 1),
                                    )
                            for i in range(2):
                                sl = slice(sh * 1024 + i * 512, sh * 1024 + (i + 1) * 512)
                                if name == "q":
                                    nc.vector.tensor_scalar_add(dst[:, sl], pjs[i], bq_sb)
                                else:
                                    nc.vector.tensor_copy(out=dst[:, sl], in_=pjs[i])
                        proj[name] = dst

                    # ---------------- phase A': v -> natural + ones col ----
                    vaug = {}
                    for hl in range(2):
                        vaug[hl] = vaugp.tile([128, TT, 65], AD, tag="vaug", name=f"vaug{hl}")
                        nc.vector.tensor_copy(
                            out=vaug[hl][:, :, 64:65], in_=ones_sb[:, 0:TT, None]
                        )
                    for tt in range(TT):
                        for hl in range(2):
                            tp = ps.tile([128, 64], AD, tag="pj", bufs=2, name="tp")
                            nc.tensor.transpose(
                                tp,
                                proj["v"][hl * 64 : hl * 64 + 64, tt * 128 : tt * 128 + 128],
                                ident[hl * 64 : hl * 64 + 64, :],
                            )
                            nc.vector.tensor_copy(out=vaug[hl][:, tt, 0:64], in_=tp)

                    # ---------------- phase B: attention ----------------
                    qt, kt = proj["q"], proj["k"]
                    for sq in range(SQ):
                        ssl = slice(sq * 512, (sq + 1) * 512)
                        at = [ps.tile([128, 512], F32, tag="at", bufs=2, name=f"at{i}") for i in range(2)]
                        for tt in range(TT):
                            tsl = slice(tt * 128, (tt + 1) * 128)
                            sc = ps.tile([128, 1024], F32, tag="sc", bufs=2, name="sc")
                            nc.tensor.matmul(
                                sc[:, 0:512], kt[0:64, tsl], qt[0:64, ssl],
                                start=True, stop=True,
                            )
                            nc.tensor.matmul(
                                sc[:, 512:1024], kt[64:128, tsl], qt[64:128, ssl],
                                start=True, stop=True,
                            )
                            ex = expp.tile([128, 1024], AD, tag="ex")
                            nc.scalar.activation(out=ex, in_=sc, func=AF.Exp, scale=0.125)
                            for hl in range(2):
                                nc.tensor.matmul(
                                    at[hl][0:65, :],
                                    vaug[hl][:, tt, :],
                                    ex[:, hl * 512 : hl * 512 + 512],
                                    start=(tt == 0),
                                    stop=(tt == TT - 1),
                                )
                        for hl in range(2):
                            u = attTs[(hl, b)]
                            nc.vector.tensor_copy(out=u[:, ssl], in_=at[hl][0:65, :])
                            with nc.allow_low_precision(reason="f32r view of f32"):
                                nc.vector.reciprocal(out=u[64:65, ssl], in_=u[64:65, ssl])



                # ---------------- phase C: normalize, then output fc ----------
                hb_list = [(hl, bb) for hl in range(2) for bb in range(B)]
                for hl, bb in hb_list:
                    u = attTs[(hl, bb)]
                    for sq in range(SQ):
                        ssl = slice(sq * 512, (sq + 1) * 512)
                        bc = ps.tile([64, 512], F32, tag="pj", bufs=2, name="bc")
                        nc.tensor.matmul(
                            bc, ones_sb[64:65, 0:64], u[64:65, ssl],
                            start=True, stop=True,
                        )
                        bcs = smallp.tile([64, 512], AD, tag="bcs")
                        nc.vector.tensor_copy(out=bcs, in_=bc)
                        nc.vector.tensor_mul(u[0:64, ssl], u[0:64, ssl], bcs)
                ysb = {
                    i: ysbp.tile([128, 1024], F32R, tag="ysb", name=f"ysb{i}")
                    for i in range(4)
                }
                for ob in range(2):
                    osl = slice(ob * 512, (ob + 1) * 512)
                    yps = [
                        ps.tile(
                            [128, 512], F32, tag=("pj" if i < 2 else "at"),
                            bufs=2, name=f"yps{i}",
                        )
                        for i in range(4)
                    ]
                    for j in range(16):
                        wo_t = wop.tile([64, 512], AD, tag="wo")
                        nc.scalar.dma_start(out=wo_t, in_=wo[j, :, osl])
                        for i, (hl, bb) in enumerate(hb_list):
                            nc.tensor.matmul(
                                yps[i],
                                attTs[(hl, bb)][0:64, j::16],
                                wo_t,
                                start=(j == 0),
                                stop=False,
                            )
                    for i, (hl, bb) in enumerate(hb_list):
                        nc.tensor.matmul(
                            yps[i],
                            ones1[0:1, 0:128],
                            bo_sb[0:1, hl, osl],
                            start=False,
                            stop=True,
                        )
                        nc.vector.tensor_copy(out=ysb[i][:, osl], in_=yps[i])
                for i, (hl, bb) in enumerate(hb_list):
                    nc.sync.dma_start(out=y[hl, bb, :, :], in_=ysb[i])

    nc.compile()
    return nc


def prep_inputs(query, key_, value, Wq, bq, Wk, bk, Wv, bv, Wo, bo):
    """Host-side sharding/packing. Returns in_maps for the 8 cores."""
    f32 = np.float32
    qT = np.ascontiguousarray(
        np.asarray(query, f32).transpose(0, 2, 1).astype(NP_AD)
    ).reshape(B, KT, 128, S)
    kT = np.ascontiguousarray(
        np.asarray(key_, f32).transpose(0, 2, 1).astype(NP_AD)
    ).reshape(B, KT, 128, S)
    vT = np.ascontiguousarray(
        np.asarray(value, f32).transpose(0, 2, 1).astype(NP_AD)
    ).reshape(B, KT, 128, S)
    Wq, Wk, Wv = (np.asarray(x, f32) for x in (Wq, Wk, Wv))
    bq, bv, Wo, bo = (np.asarray(x, f32) for x in (bq, bv, Wo, bo))
    wo_r = np.ascontiguousarray(Wo.reshape(16, 64, 1024).astype(NP_AD))
    eye = np.ascontiguousarray(np.concatenate([np.eye(64, dtype=NP_AD)] * 2))
    ones = np.ones((128, 128), dtype=NP_AD)
    in_maps = []
    for c in range(NCORES):
        h0, h1 = 2 * c, 2 * c + 1

        def pack(w):
            return np.ascontiguousarray(
                np.concatenate([w[h0], w[h1]], axis=1)
                .reshape(KT, 128, 128)
                .transpose(1, 0, 2)
                .astype(NP_AD)
            )

        bq_p = np.ascontiguousarray(
            np.concatenate([bq[h0], bq[h1]]).reshape(128, 1).astype(f32)
        )
        bo2 = np.ascontiguousarray(
            np.stack(
                [bo + np.tile(bv[h], H) @ Wo for h in (h0, h1)]
            ).astype(NP_AD)
        )
        in_maps.append(
            dict(
                qT=qT, kT=kT, vT=vT,
                wq=pack(Wq), wk=pack(Wk), wv=pack(Wv),
                bq=bq_p, wo=wo_r, bo2=bo2, eye=eye, ones=ones,
            )
        )
    return in_maps


def assemble_output(results):
    out = np.empty((H * 256, 1024), np.float32)
    for c in range(NCORES):
        yc = results[c]["y"]  # [2, B, 128, 1024]
        for hl in range(2):
            h = 2 * c + hl
            out[h * 256 : (h + 1) * 256] = yc[hl].reshape(256, 1024)
    return out.reshape(B, S, D)


_NC_CACHE = {}


def run(inputs, trace=False, reps=1):
    from concourse.bass_utils import run_bass_kernel_spmd

    if reps not in _NC_CACHE:
        _NC_CACHE[reps] = build_nc(reps)
    nc = _NC_CACHE[reps]
    in_maps = prep_inputs(**inputs)
    r = run_bass_kernel_spmd(nc, in_maps, list(range(NCORES)), trace=trace)
    return assemble_output(r.results), r


def kernel(**inputs) -> np.ndarray:
    out, _ = run(inputs, trace=False)
    return out

